# revision 25
# baseline (speedup 1.0000x reference)
"""Trainium2 Bass kernel for nn_AxialBlock (3-axis axial attention sum).

Problem (hardcoded): x (B=4, C=512, T=16, H=32, W=32) fp32, three axial
MHA blocks (attend along W, H, T; n_head=8, d=64) each with their own
QKVO projections; outputs summed. Output (B, C, T, H, W) fp32.

Sharding: 8 cores, every pass fully local (no collectives, no recompute):
  - w-pass / t-pass: core (b, j) owns H-half j of sample b; rows along W
    (w-fastest order) resp. packed T-fiber pairs (t-fastest order).
  - h-pass: core (b, j) owns W-half j of sample b (all H present), rows
    along H (h-fastest order).
Because the h-pass token set differs from the w/t set, the kernel emits
TWO partial outputs per core: y_wt = w+t passes (+ summed bias) over the
(b, H-half) tokens, and y_h over the (b, W-half) tokens. The host adds
the overlapping quadrants (attention outputs sum, so this is exact).

On-device layout trick: x is channels-first, i.e. already "x^T" (C on
partitions) which is what the PE wants for the QKV projections. The host
pre-permutes x into three token orders (w-fastest / t-fastest / h-fastest)
so that each axial attention acts on 32 consecutive tokens ("rows").

Per 512-token tile (16 rows x 32 tokens):
  q^T (feat-partition) and v (token-partition) projections in bf16; k is
  evacuated parity-split into persistent pre-zeroed "kz" buffers (one head
  per 64 d-rows, rest zero) so attention scores can contract over all 128
  partitions - the PE array tiling positions with BOTH row!=0 and col!=0
  hard-crash the device (NRT_EXEC_UNIT_UNRECOVERABLE), so only (0, col) /
  (row, 0) tiles are usable. Scores: one (K=128, M=32, N=64) matmul per
  (chunk, row) computing both heads of the chunk at col-tile (0, 32j).
  (A 64-token-row variant with half the S matmuls was tried and measured
  SLOWER on HW - 1.62ms vs 1.32ms - despite the ~120ns/matmul issue
  floor; the doubled softmax volume and op count dominated. Keep rows=32.)
  Softmax is batched per 2 row groups with one op per step: exp on
  ScalarE, per-fiber-block reduce + reciprocal on VectorE (for t, the
  reciprocal is multiplied by a 0/1 fiber mask, zeroing cross-fiber
  weights exactly - replaces the old additive -60 mask matmuls), and the
  broadcast normalize on GpSimd (measured ~4x faster there than DVE).
  A -> A^T via the full-width DVE 32x32 block transpose, then per-row
  contiguous (32, 512) DVE copies form a block-diagonal A^T ("abd") in
  persistent zeroed double buffers; o^T = V^T @ abd lands feat-partition
  directly as one (K=128, M=64, N=128) matmul per (chunk, group,
  head-parity); then the out-projection.

Output accumulation is SBUF-resident (no DRAM read-modify-write): the
w-pass writes its out-projection (+ summed bias) into a persistent bf16
y_sb buffer; the t-pass adds its out-projection to the matching strided
y_sb view on DVE and emits the sum contiguously (tile order) to y_wt.
The h-pass emits directly to y_h. Host de-permutes + upcasts + adds.
Pass order w -> h -> t so the t-pass's read-after-write dependency on
y_sb never stalls the pipeline.

t-axis has seq len 16: two t-fibers are packed into one 32-token row with
the fiber-masked reciprocal zeroing cross-fiber attention.
"""

import contextlib

import ml_dtypes
import numpy as np

import concourse.bass as bass
import concourse.tile as tile
from concourse import bacc, mybir
from concourse.bass_utils import run_bass_kernel_spmd

BF16 = mybir.dt.bfloat16
FP32 = mybir.dt.float32
BF16_NP = np.dtype(ml_dtypes.bfloat16)

B, C, T, H, W = 4, 512, 16, 32, 32
NH, D = 8, 64
HL = H // 2              # per-core H slice (w/t passes)
WL = W // 2              # per-core W slice (h-pass)
N_CORES = 8
TOK_LOCAL = T * HL * W   # 8192 tokens owned per core per pass
TILE = 512               # tokens per on-chip tile
NCH = C // 128           # 4 partition chunks of the feature dim

# dev knob: cap tiles per pass (None = full problem). Truncated builds are
# only for fast AP/scheduling smoke tests - output is wrong when set.
NTILES_CAP = None
# dev knob: repeat the whole workload K times (output stays correct: w-pass
# overwrites y_sb, t/h emit fresh tiles each rep).
REPS = 1
# dev knob: ablations for HW time attribution (output wrong when set):
#   "attn"    - skip S matmuls, softmax and O matmuls (out-proj reads v)
#   "softmax" - keep S and O matmuls, skip the softmax/transpose chain
ABLATE = None


def _build_pass(tc, pools, axis, x_ap, w_aps, y_ap, bias_aps, fm_sb,
                kz_tiles, abd_tiles, y_sb):
    """Emit one axial-attention pass.

    axis: 'w' | 't' | 'h'.  x_ap: (512, 8192) bf16 DRAM, token order chosen
    so each 32-token group is one attention row.  y_ap: (512, 8192) bf16
    DRAM output (y_wt for 't', y_h for 'h'; unused for 'w' which writes
    the persistent y_sb SBUF accumulator instead).
    """
    nc = tc.nc
    wq_sb, wk_sb, wv_sb, wo_sb = w_aps
    ntiles = TOK_LOCAL // TILE
    if NTILES_CAP is not None:
        ntiles = min(ntiles, NTILES_CAP)

    (xt_pool, qk_pool, v_pool, a_pool, sm_pool,
     ot_pool, y_pool, ps_pool, sps_pool) = pools

    def emit_front(it):
        """Loads, q/k projections, S matmuls + pair-batched softmax; the
        v projection is emitted mid-S so the second S pair's wait on the
        (single, 2-bank) score psum's readers is hidden behind it.
        Returns the state the back stage (O + out-projection) needs."""
        # ---- load x^T tile: (128, NCH, TILE) bf16, free = (chunk, token)
        xt = xt_pool.tile([128, NCH, TILE], BF16)
        for kc in range(NCH):
            nc.sync.dma_start(
                xt[:, kc, :], x_ap[128 * kc:128 * (kc + 1), it * TILE:(it + 1) * TILE]
            )

        # ---- q^T, k^T projections: feat-partition bf16.
        # k is evacuated parity-split straight into the persistent pre-zeroed
        # kz buffers (head p's 64 d-rows in place, other 64 rows zero), so
        # the S matmul can contract over all 128 partitions — the only legal
        # PE tile positions are row 0 / col 0 (see module docstring).
        q_sb = qk_pool.tile([128, NCH, TILE], BF16, tag="q")
        kz_sb = kz_tiles[tc._kz_flip]
        tc._kz_flip ^= 1
        for w_sb, ev in ((wq_sb, 0), (wk_sb, 1)):
            for mc in range(NCH):
                ps = ps_pool.tile([128, TILE], FP32, tag="ps", bufs=2)
                for kc in range(NCH):
                    nc.tensor.matmul(
                        ps[:],
                        lhsT=w_sb[:, kc, 128 * mc:128 * (mc + 1)],
                        rhs=xt[:, kc, :],
                        start=(kc == 0), stop=(kc == NCH - 1),
                    )
                if ev == 0:
                    nc.scalar.copy(q_sb[:, mc, :], ps[:])
                elif mc < 2:
                    nc.scalar.copy(kz_sb[0:64, 0, mc, :], ps[0:64, :])
                    nc.scalar.copy(kz_sb[64:128, 1, mc, :], ps[64:128, :])
                else:
                    nc.vector.tensor_copy(kz_sb[0:64, 0, mc, :], ps[0:64, :])
                    nc.vector.tensor_copy(kz_sb[64:128, 1, mc, :], ps[64:128, :])

        v_sb = v_pool.tile([128, NCH, C], BF16)

        def emit_vproj():
            # v projection, token-partition: (128, NCH, C) bf16
            for ts in range(NCH):
                ps = ps_pool.tile([128, TILE], FP32, tag="ps", bufs=2)
                for kc in range(NCH):
                    nc.tensor.matmul(
                        ps[:],
                        lhsT=xt[:, kc, 128 * ts:128 * (ts + 1)],
                        rhs=wv_sb[:, kc, :],
                        start=(kc == 0), stop=(kc == NCH - 1),
                    )
                if ts % 2 == 0:
                    nc.scalar.copy(v_sb[:, ts, :], ps[:])
                else:
                    nc.vector.tensor_copy(v_sb[:, ts, :], ps[:])

        abdpairs = abd_tiles[tc._abd_flip]
        tc._abd_flip ^= 1
        st = {"it": it, "v_sb": v_sb, "abdpairs": abdpairs}
        if ABLATE == "attn":
            emit_vproj()
            return st

        # ---- attention at 64-token-row granularity (rows pack the axis
        # fibers; w/h: 2 fibers of 32, t: 4 fibers of 16). Per group g
        # (128 tokens = 2 rows j2 of 64): scores at (j2*64+q, gm-bank-half,
        # c*128 + par*64 + key); one matmul per (chunk, row) - half the S
        # matmul count of 32-token rows (small PE matmuls pay a fixed
        # ~128-cycle weight-load each, measured).
        nf = 4 if axis == "t" else 2     # fibers per 32-key block granularity
        a_by_pair = {}
        for pr in range(2):
            sps = sps_pool.tile([128, 2, 512], FP32, name="sps", bufs=1)
            a_pr = a_pool.tile([128, 1024], BF16, tag="a")
            for gm in range(2):
                g = 2 * pr + gm
                for c in range(NCH):
                    for j2 in range(2):
                        tok0 = (g * 2 + j2) * 64
                        nc.tensor.matmul(
                            sps[64 * j2:64 * j2 + 64, gm, 128 * c:128 * (c + 1)],
                            lhsT=q_sb[:, c, tok0:tok0 + 64],
                            rhs=kz_sb[:, :, c, tok0:tok0 + 64],
                            tile_position=(0, 64 * j2),
                            start=True, stop=False, skip_group_check=True,
                        )
            for gm in range(2):
                nc.scalar.activation(a_pr[:, 512 * gm:512 * (gm + 1)],
                                     sps[:, gm, :],
                                     mybir.ActivationFunctionType.Exp)
            a_by_pair[pr] = a_pr
            if pr == 0:
                emit_vproj()

        if ABLATE == "softmax":
            return st

        # ---- pair-batched softmax: ONE op per step covers both groups of
        # the pair (2048 score columns). Cross-fiber garbage is finite; it
        # is either never copied into abd (32-token fibers == the 32-wide
        # transpose blocks) or zeroed by the fiber-masked reciprocal (t).
        for pr in range(2):
            a_pr = a_by_pair[pr]
            a4 = a_pr[:].rearrange("p (G f k) -> p G f k", G=2 * NH, f=nf)
            sums = sm_pool.tile([128, 2 * NH * nf], FP32, tag="sums")
            nc.vector.tensor_reduce(
                sums[:], a4, axis=mybir.AxisListType.X, op=mybir.AluOpType.add
            )
            recip = sm_pool.tile([128, 2 * NH * nf], FP32, tag="recip")
            nc.vector.reciprocal(recip[:], sums[:])
            if nf == 4:
                recipm = sm_pool.tile([128, 2 * NH * nf], FP32, tag="recipm")
                nc.vector.tensor_tensor(
                    recipm[:].rearrange("p (G f) -> p G f", G=2 * NH),
                    recip[:].rearrange("p (G f) -> p G f", G=2 * NH),
                    fm_sb[:].unsqueeze(1).broadcast_to((128, 2 * NH, nf)),
                    mybir.AluOpType.mult,
                )
                recip = recipm
            nc.gpsimd.tensor_tensor(
                a4, a4,
                recip[:].rearrange("p (G f) -> p G f", G=2 * NH)
                .unsqueeze(3).broadcast_to((128, 2 * NH, nf, 64 // nf)),
                mybir.AluOpType.mult,
            )
            # A -> A^T (DVE 32x32 block transpose over the full pair), then
            # 4 same-fiber block copies assemble the pair's block-diagonal
            # A^T: abd[(j2,ksub) block, gm, slot, j2*64+ksub*32+q] <-
            # at[(j2,ksub) block, (gm, slot, ksub) cols]. Cross-fiber and
            # cross-row positions stay zero from the one-time memset.
            at_pr = a_pool.tile([128, 1024], BF16, tag="at")
            nc.vector.transpose(at_pr[:], a_pr[:])
            abdp = abdpairs[pr]
            for j2 in range(2):
                for ksub in range(2):
                    sp = 32 * (2 * j2 + ksub)
                    src = (at_pr[sp:sp + 32, :]
                           .rearrange("p (m s u) -> p m s u", m=2, s=2 * NCH)
                           [:, :, :, 32 * ksub:32 * ksub + 32])
                    dst = abdp[sp:sp + 32, :, :,
                               64 * j2 + 32 * ksub:64 * j2 + 32 * ksub + 32]
                    if (j2 + ksub) % 2 == 0:
                        nc.vector.tensor_copy(dst, src)
                    else:
                        nc.gpsimd.tensor_copy(dst, src)
        return st

    def emit_back(st):
        """O matmuls + out-projection + output emission for tile st (one
        tile behind the front stage, so the softmax chains feeding abd are
        long since complete when the O matmuls issue)."""
        it = st["it"]
        v_sb = st["v_sb"]
        abdpairs = st["abdpairs"]
        # ---- o^T = V^T A_bd: one (K=128, M=64, N=128) matmul per (chunk,
        # group, head-parity); chunk-outer, evacuated to bf16 per chunk
        ot_sb = ot_pool.tile([128, NCH, TILE], BF16)
        if ABLATE == "attn":
            for c in range(NCH):
                nc.gpsimd.tensor_copy(ot_sb[:, c, :], v_sb[:, c, :])
        else:
            for c in range(NCH):
                otp = ps_pool.tile([128, TILE], FP32, name="otp", tag="otp",
                                   bufs=2)
                for g in range(4):
                    pr, gm = divmod(g, 2)
                    for p in range(2):
                        nc.tensor.matmul(
                            otp[64 * p:64 * (p + 1), 128 * g:128 * (g + 1)],
                            lhsT=v_sb[:, g,
                                      (2 * c + p) * 64:(2 * c + p + 1) * 64],
                            rhs=abdpairs[pr][:, gm, 2 * c + p, :],
                            tile_position=(0, 64 * p),
                        )
                if c % 2 == 0:
                    nc.scalar.copy(ot_sb[:, c, :], otp[:])
                else:
                    nc.vector.tensor_copy(ot_sb[:, c, :], otp[:])

        # ---- out-projection + emit / accumulate
        for mc in range(NCH):
            yps = ps_pool.tile([128, TILE], FP32, name="yps", tag="yps", bufs=2)
            for kc in range(NCH):
                nc.tensor.matmul(
                    yps[:],
                    lhsT=wo_sb[:, kc, 128 * mc:128 * (mc + 1)],
                    rhs=ot_sb[:, kc, :],
                    start=(kc == 0), stop=(kc == NCH - 1),
                )
            cs = slice(128 * mc, 128 * (mc + 1))
            if axis == "w":
                # first pass: write into the persistent SBUF accumulator,
                # folding the (summed) output bias in. w tile it covers
                # t=it, so this is the contiguous y_sb token range.
                nc.scalar.activation(
                    y_sb[:, mc, it * TILE:(it + 1) * TILE], yps[:],
                    mybir.ActivationFunctionType.Identity,
                    bias=bias_aps[mc],
                )
            elif axis == "t":
                # t tile it covers h-row `it`; psum tokens are (w 32, t 16)
                # t-fastest. Add the matching strided y_sb view (token order
                # (t, hl, w)) and emit contiguously in tile order.
                yv = (y_sb[:, mc, :].rearrange("p (t h w) -> p t h w",
                                               t=T, h=HL, w=W)[:, :, it, :]
                      .transpose([0, 2, 1]))                    # (128, w, t)
                ynew = y_pool.tile([128, W, T], BF16, tag="yt")
                yp3 = yps[:].rearrange("p (w t) -> p w t", w=W)
                nc.vector.tensor_tensor(
                    ynew[:], yv, yp3, mybir.AluOpType.add
                )
                nc.sync.dma_start(y_ap[cs, it * TILE:(it + 1) * TILE], ynew[:])
            else:
                # h-pass: independent token set; emit directly (no bias).
                y_sb2 = y_pool.tile([128, TILE], BF16, tag="yh")
                if mc % 2 == 0:
                    nc.scalar.copy(y_sb2[:], yps[:])
                else:
                    nc.vector.tensor_copy(y_sb2[:], yps[:])
                nc.sync.dma_start(y_ap[cs, it * TILE:(it + 1) * TILE], y_sb2[:])

    prev = None
    for it in range(ntiles):
        st = emit_front(it)
        if prev is not None:
            emit_back(prev)
        prev = st
    if prev is not None:
        emit_back(prev)


def build_program():
    """Build + compile the SPMD bass program (same program on all 8 cores)."""
    nc = bacc.Bacc(
        "TRN2", target_bir_lowering=False, debug=False,
        enable_asserts=False, num_devices=N_CORES,
    )

    def din(name, shape, dt=BF16):
        return nc.dram_tensor(name, shape, dt, kind="ExternalInput").ap()

    x_w = din("x_w", (C, TOK_LOCAL))
    x_t = din("x_t", (C, TOK_LOCAL))
    x_h = din("x_h", (C, TOK_LOCAL))
    w_in = {}
    for ax in ("w", "t", "h"):
        for nm in ("wq", "wk", "wv", "wo"):
            w_in[f"{nm}_{ax}"] = din(f"{nm}_{ax}", (C, C))
    bias_in = din("bias", (C, 1), FP32)
    fm2_in = din("fm2", (128, 4), FP32)
    y_wt = nc.dram_tensor("y_wt", (C, TOK_LOCAL), BF16, kind="ExternalOutput").ap()
    y_h = nc.dram_tensor("y_h", (C, TOK_LOCAL), BF16, kind="ExternalOutput").ap()

    with tile.TileContext(nc) as tc:
        with contextlib.ExitStack() as ctx:
            xt_pool = ctx.enter_context(tc.tile_pool(name="xt", bufs=3))
            w_pool = ctx.enter_context(tc.tile_pool(name="wts", bufs=2))
            qk_pool = ctx.enter_context(tc.tile_pool(name="qk", bufs=2))
            v_pool = ctx.enter_context(tc.tile_pool(name="v", bufs=2))
            a_pool = ctx.enter_context(tc.tile_pool(name="a", bufs=3))
            sm_pool = ctx.enter_context(tc.tile_pool(name="sm", bufs=3))
            ot_pool = ctx.enter_context(tc.tile_pool(name="ot", bufs=2))
            y_pool = ctx.enter_context(tc.tile_pool(name="y", bufs=3))
            ps_pool = ctx.enter_context(tc.tile_pool(name="ps", bufs=2, space="PSUM"))
            sps_pool = ctx.enter_context(tc.tile_pool(name="sps", bufs=2, space="PSUM"))
            const_pool = ctx.enter_context(tc.tile_pool(name="const", bufs=1))

            # constants
            fm2_sb = const_pool.tile([128, 4], FP32)
            nc.sync.dma_start(fm2_sb[:], fm2_in[:])
            bias_sb = const_pool.tile([128, NCH], FP32)
            for mc in range(NCH):
                nc.sync.dma_start(
                    bias_sb[:, mc:mc + 1], bias_in[128 * mc:128 * (mc + 1), :]
                )
            bias_aps = [bias_sb[:, mc:mc + 1] for mc in range(NCH)]

            # persistent SBUF output accumulator for the w+t passes
            y_sb = const_pool.tile([128, NCH, TOK_LOCAL], BF16, name="y_sb")

            # persistent block-diagonal A^T buffers: per (tile-parity, group
            # pair) a (128 = (j2, key) token-in-group, 2 gm, NH slots, 128 =
            # (j2, q) query-in-group) tile, zeroed once (cross-row and
            # cross-fiber blocks stay zero forever)
            abd_tiles = []
            for i in range(2):
                prs = []
                for pr in range(2):
                    t = const_pool.tile([128, 2, NH, 128], BF16,
                                        name=f"abd{i}_{pr}")
                    nc.gpsimd.memset(t[:], 0.0)
                    prs.append(t)
                abd_tiles.append(prs)
            tc._abd_flip = 0
            kz_tiles = []
            for i in range(2):
                t = const_pool.tile([128, 2, NCH, TILE], BF16, name=f"kz{i}")
                nc.gpsimd.memset(t[:], 0.0)
                kz_tiles.append(t)
            tc._kz_flip = 0

            pools = (xt_pool, qk_pool, v_pool, a_pool, sm_pool,
                     ot_pool, y_pool, ps_pool, sps_pool)

            for _rep in range(REPS):
              for ax, x_ap, y_ap in (("w", x_w, None), ("h", x_h, y_h),
                                     ("t", x_t, y_wt)):
                w_aps = []
                for nm in ("wq", "wk", "wv", "wo"):
                    wt = w_pool.tile([128, NCH, C], BF16, tag=nm, name=nm)
                    for kc in range(NCH):
                        nc.sync.dma_start(
                            wt[:, kc, :],
                            w_in[f"{nm}_{ax}"][128 * kc:128 * (kc + 1), :],
                        )
                    w_aps.append(wt)
                _build_pass(tc, pools, ax, x_ap, w_aps, y_ap, bias_aps, fm2_sb,
                            kz_tiles, abd_tiles, y_sb)

    nc.compile()
    return nc


_PROGRAM = None


def _get_program():
    global _PROGRAM
    if _PROGRAM is None:
        _PROGRAM = build_program()
    return _PROGRAM


def make_in_maps(inputs):
    """Host-side shard + layout prep: per-core input dicts."""
    x = np.asarray(inputs["x"], np.float32)          # (B, C, T, H, W)
    scale = 1.0 / np.sqrt(D)

    weights = {}
    for ax in ("w", "h", "t"):
        for nm in ("wq", "wk", "wv", "wo"):
            wm = np.asarray(inputs[f"{nm}_{ax}"], np.float32)
            if nm == "wq":
                wm = wm * scale
            # lhsT layout: (C_in, C_out) = W.T
            weights[f"{nm}_{ax}"] = np.ascontiguousarray(wm.T).astype(BF16_NP)
    bias = (np.asarray(inputs["bo_w"], np.float32)
            + np.asarray(inputs["bo_h"], np.float32)
            + np.asarray(inputs["bo_t"], np.float32)).reshape(C, 1)

    # t-pass fiber mask for the masked-reciprocal softmax: fm[p, f] = 1 iff
    # query partition p (= j2*64 + q) belongs to 16-token fiber f of its
    # 64-token row
    p = np.arange(128) % 64
    fm2 = np.stack([(p // 16) == f for f in range(4)], axis=1).astype(np.float32)

    in_maps = []
    for core in range(N_CORES):
        b, j = divmod(core, 2)
        xb = x[b]                                    # (C, T, H, W)
        xw = xb[:, :, 16 * j:16 * (j + 1), :]        # (C, T, HL, W) w-fastest
        xt = np.transpose(xw, (0, 2, 3, 1))          # (C, HL, W, T) t-fastest
        xh = np.transpose(xb[:, :, :, 16 * j:16 * (j + 1)],
                          (0, 1, 3, 2))              # (C, T, WL, H) h-fastest
        m = {
            "x_w": np.ascontiguousarray(xw).reshape(C, TOK_LOCAL).astype(BF16_NP),
            "x_t": np.ascontiguousarray(xt).reshape(C, TOK_LOCAL).astype(BF16_NP),
            "x_h": np.ascontiguousarray(xh).reshape(C, TOK_LOCAL).astype(BF16_NP),
            "bias": bias, "fm2": fm2,
        }
        m.update(weights)
        in_maps.append(m)
    return in_maps


def assemble_output(results):
    """Gather per-core y_wt/y_h partials into (B, C, T, H, W) fp32."""
    out = np.empty((B, C, T, H, W), np.float32)
    for core in range(N_CORES):
        b, j = divmod(core, 2)
        # y_wt tiles are h-rows: (C, HL, W, T) -> (C, T, HL, W)
        ywt = np.asarray(results[core]["y_wt"], np.float32).reshape(C, HL, W, T)
        out[b, :, :, 16 * j:16 * (j + 1), :] = np.transpose(ywt, (0, 3, 1, 2))
    for core in range(N_CORES):
        b, j = divmod(core, 2)
        # y_h tiles are t-slices: (C, T, WL, H) -> (C, T, H, WL)
        yh = np.asarray(results[core]["y_h"], np.float32).reshape(C, T, WL, H)
        out[b, :, :, :, 16 * j:16 * (j + 1)] += np.transpose(yh, (0, 1, 3, 2))
    return out


_RUNNER = None


def _get_runner():
    """Build the sharded PJRT callable once; reuse across kernel() calls."""
    global _RUNNER
    if _RUNNER is not None:
        return _RUNNER
    import jax
    from jax.sharding import Mesh, PartitionSpec
    from jax.experimental.shard_map import shard_map
    from concourse import bass2jax

    nc = _get_program()
    bass2jax.install_neuronx_cc_hook()
    partition_name = (nc.partition_id_tensor.name
                      if nc.partition_id_tensor else None)
    in_names, out_names, out_avals, zero_outs = [], [], [], []
    for alloc in nc.m.functions[0].allocations:
        if not isinstance(alloc, mybir.MemoryLocationSet):
            continue
        name = alloc.memorylocations[0].name
        if alloc.kind == "ExternalInput":
            if name != partition_name:
                in_names.append(name)
        elif alloc.kind == "ExternalOutput":
            out_names.append(name)
            shape = tuple(alloc.tensor_shape)
            dtype = mybir.dt.np(alloc.dtype)
            out_avals.append(jax.core.ShapedArray(shape, dtype))
            zero_outs.append(np.zeros((N_CORES * shape[0], *shape[1:]), dtype))
    n_params = len(in_names)
    all_in_names = list(in_names) + out_names
    if partition_name is not None:
        all_in_names.append(partition_name)

    def _body(*args):
        operands = list(args)
        if partition_name is not None:
            operands.append(bass2jax.partition_id_tensor())
        return tuple(bass2jax._bass_exec_p.bind(
            *operands,
            out_avals=tuple(out_avals),
            in_names=tuple(all_in_names),
            out_names=tuple(out_names),
            lowering_input_output_aliases=(),
            sim_require_finite=True,
            sim_require_nnan=True,
            nc=nc,
        ))

    devices = jax.devices()[:N_CORES]
    mesh = Mesh(np.asarray(devices), ("core",))
    in_specs = (PartitionSpec("core"),) * (n_params + len(out_names))
    out_specs = (PartitionSpec("core"),) * len(out_names)
    fn = jax.jit(shard_map(_body, mesh=mesh, in_specs=in_specs,
                           out_specs=out_specs, check_rep=False))

    def run(in_maps):
        concat_in = [
            np.concatenate([np.asarray(in_maps[c][nm]) for c in range(N_CORES)],
                           axis=0)
            for nm in in_names
        ]
        outs = fn(*concat_in, *zero_outs)
        return [
            {nm: np.asarray(outs[i]).reshape(N_CORES, *out_avals[i].shape)[c]
             for i, nm in enumerate(out_names)}
            for c in range(N_CORES)
        ]

    _RUNNER = run
    return run


def kernel(**inputs) -> np.ndarray:
    run = _get_runner()
    in_maps = make_in_maps(inputs)
    return assemble_output(run(in_maps))


# revision 26
# speedup vs baseline: 1.1816x; 1.1816x over previous
"""Trainium2 Bass kernel for nn_AxialBlock (3-axis axial attention sum).

Problem (hardcoded): x (B=4, C=512, T=16, H=32, W=32) fp32, three axial
MHA blocks (attend along W, H, T; n_head=8, d=64) each with their own
QKVO projections; outputs summed. Output (B, C, T, H, W) fp32.

Sharding: 8 cores, every pass fully local (no collectives, no recompute):
  - w-pass / t-pass: core (b, j) owns H-half j of sample b; rows along W
    (w-fastest order) resp. packed T-fiber pairs (t-fastest order).
  - h-pass: core (b, j) owns W-half j of sample b (all H present), rows
    along H (h-fastest order).
Because the h-pass token set differs from the w/t set, the kernel emits
TWO partial outputs per core: y_wt = w+t passes (+ summed bias) over the
(b, H-half) tokens, and y_h over the (b, W-half) tokens. The host adds
the overlapping quadrants (attention outputs sum, so this is exact).

On-device layout trick: x is channels-first, i.e. already "x^T" (C on
partitions) which is what the PE wants for the QKV projections. The host
pre-permutes x into three token orders (w-fastest / t-fastest / h-fastest)
so that each axial attention acts on 32 consecutive tokens ("rows").

Per 512-token tile (16 rows x 32 tokens):
  q^T (feat-partition) and v (token-partition) projections in bf16; k is
  evacuated parity-split into persistent pre-zeroed "kz" buffers (one head
  per 64 d-rows, rest zero) so attention scores can contract over all 128
  partitions - the PE array tiling positions with BOTH row!=0 and col!=0
  hard-crash the device (NRT_EXEC_UNIT_UNRECOVERABLE), so only (0, col) /
  (row, 0) tiles are usable. Scores: one (K=128, M=32, N=64) matmul per
  (chunk, row) computing both heads of the chunk at col-tile (0, 32j).
  (A 64-token-row variant with half the S matmuls was tried and measured
  SLOWER on HW - 1.62ms vs 1.32ms - despite the ~120ns/matmul issue
  floor; the doubled softmax volume and op count dominated. Keep rows=32.)
  Softmax is batched per 2 row groups with one op per step: exp on
  ScalarE, per-fiber-block reduce + reciprocal on VectorE (for t, the
  reciprocal is multiplied by a 0/1 fiber mask, zeroing cross-fiber
  weights exactly - replaces the old additive -60 mask matmuls), and the
  broadcast normalize on GpSimd (measured ~4x faster there than DVE).
  A -> A^T via the full-width DVE 32x32 block transpose, then per-row
  contiguous (32, 512) DVE copies form a block-diagonal A^T ("abd") in
  persistent zeroed double buffers; o^T = V^T @ abd lands feat-partition
  directly as one (K=128, M=64, N=128) matmul per (chunk, group,
  head-parity); then the out-projection.

Output accumulation is SBUF-resident (no DRAM read-modify-write): the
w-pass writes its out-projection (+ summed bias) into a persistent bf16
y_sb buffer; the t-pass adds its out-projection to the matching strided
y_sb view on DVE and emits the sum contiguously (tile order) to y_wt.
The h-pass emits directly to y_h. Host de-permutes + upcasts + adds.
Pass order w -> h -> t so the t-pass's read-after-write dependency on
y_sb never stalls the pipeline.

t-axis has seq len 16: two t-fibers are packed into one 32-token row with
the fiber-masked reciprocal zeroing cross-fiber attention.
"""

import contextlib

import ml_dtypes
import numpy as np

import concourse.bass as bass
import concourse.tile as tile
from concourse import bacc, mybir
from concourse.bass_utils import run_bass_kernel_spmd

BF16 = mybir.dt.bfloat16
FP32 = mybir.dt.float32
BF16_NP = np.dtype(ml_dtypes.bfloat16)

B, C, T, H, W = 4, 512, 16, 32, 32
NH, D = 8, 64
HL = H // 2              # per-core H slice (w/t passes)
WL = W // 2              # per-core W slice (h-pass)
N_CORES = 8
TOK_LOCAL = T * HL * W   # 8192 tokens owned per core per pass
TILE = 512               # tokens per on-chip tile
NCH = C // 128           # 4 partition chunks of the feature dim

# dev knob: cap tiles per pass (None = full problem). Truncated builds are
# only for fast AP/scheduling smoke tests - output is wrong when set.
NTILES_CAP = None
# dev knob: repeat the whole workload K times (output stays correct: w-pass
# overwrites y_sb, t/h emit fresh tiles each rep).
REPS = 1
# dev knob: ablations for HW time attribution (output wrong when set):
#   "attn"    - skip S matmuls, softmax and O matmuls (out-proj reads v)
#   "softmax" - keep S and O matmuls, skip the softmax/transpose chain
ABLATE = None


def _build_pass(tc, pools, axis, x_ap, w_aps, y_ap, bias_aps, fm_sb,
                kz_tiles, abd_tiles, y_sb):
    """Emit one axial-attention pass.

    axis: 'w' | 't' | 'h'.  x_ap: (512, 8192) bf16 DRAM, token order chosen
    so each 32-token group is one attention row.  y_ap: (512, 8192) bf16
    DRAM output (y_wt for 't', y_h for 'h'; unused for 'w' which writes
    the persistent y_sb SBUF accumulator instead).
    """
    nc = tc.nc
    wq_sb, wk_sb, wv_sb, wo_sb = w_aps
    ntiles = TOK_LOCAL // TILE
    if NTILES_CAP is not None:
        ntiles = min(ntiles, NTILES_CAP)

    (xt_pool, qk_pool, v_pool, a_pool, sm_pool,
     ot_pool, y_pool, ps_pool, sps_pool) = pools

    for it in range(ntiles):
        # ---- load x^T tile: (128, NCH, TILE) bf16, free = (chunk, token)
        xt = xt_pool.tile([128, NCH, TILE], BF16)
        for kc in range(NCH):
            nc.sync.dma_start(
                xt[:, kc, :], x_ap[128 * kc:128 * (kc + 1), it * TILE:(it + 1) * TILE]
            )

        # ---- q^T, k^T projections: feat-partition bf16.
        # k is evacuated parity-split straight into the persistent pre-zeroed
        # kz buffers (head p's 64 d-rows in place, other 64 rows zero), so
        # the S matmul can contract over all 128 partitions — the only legal
        # PE tile positions are row 0 / col 0 (see module docstring).
        q_sb = qk_pool.tile([128, NCH, TILE], BF16, tag="q")
        kz_sb = kz_tiles[tc._kz_flip]
        tc._kz_flip ^= 1
        for w_sb, ev in ((wq_sb, 0), (wk_sb, 1)):
            for mc in range(NCH):
                ps = ps_pool.tile([128, TILE], FP32, tag="ps", bufs=2)
                for kc in range(NCH):
                    nc.tensor.matmul(
                        ps[:],
                        lhsT=w_sb[:, kc, 128 * mc:128 * (mc + 1)],
                        rhs=xt[:, kc, :],
                        start=(kc == 0), stop=(kc == NCH - 1),
                    )
                if ev == 0:
                    nc.scalar.copy(q_sb[:, mc, :], ps[:])
                elif mc < 2:
                    nc.scalar.copy(kz_sb[0:64, 0, mc, :], ps[0:64, :])
                    nc.scalar.copy(kz_sb[64:128, 1, mc, :], ps[64:128, :])
                else:
                    nc.vector.tensor_copy(kz_sb[0:64, 0, mc, :], ps[0:64, :])
                    nc.vector.tensor_copy(kz_sb[64:128, 1, mc, :], ps[64:128, :])

        # ---- v projection, token-partition: (128, NCH, C) bf16,
        #      free = (token block ts, feature)
        v_sb = v_pool.tile([128, NCH, C], BF16)
        for ts in range(NCH):
            ps = ps_pool.tile([128, TILE], FP32, tag="ps", bufs=2)
            for kc in range(NCH):
                nc.tensor.matmul(
                    ps[:],
                    lhsT=xt[:, kc, 128 * ts:128 * (ts + 1)],
                    rhs=wv_sb[:, kc, :],
                    start=(kc == 0), stop=(kc == NCH - 1),
                )
            if ts % 2 == 0:
                nc.scalar.copy(v_sb[:, ts, :], ps[:])
            else:
                nc.vector.tensor_copy(v_sb[:, ts, :], ps[:])

        # ---- attention: 16 rows x 8 heads of 32x32 blocks
        qm = 32                          # query rows per 32-token row
        GW = NH * 32                     # 256 free columns per row group
        nf = 2 if axis == "t" else 1     # fibers per 32-token row
        abd_by_g = {}
        if ABLATE != "attn":
            # ---- scores + softmax at 2-rowgroup granularity: S psum
            # (128, 512) = one bank; free = (g%2)*256 + head-slot*32 + kpos.
            # One matmul per (chunk, row) computes BOTH heads of the chunk:
            # the moving operand stacks kz[par=0] and kz[par=1] columns
            # (N=64), sharing a single q stationary load.
            for gg in range(2):
                sps = sps_pool.tile([128, 2 * GW], FP32)
                for gh in range(2):
                    g = 2 * gg + gh
                    for c in range(NCH):
                        for j in range(4):
                            qcol = (g * 4 + j) * qm
                            nc.tensor.matmul(
                                sps[32 * j:32 * j + qm,
                                    gh * GW + 2 * c * 32:gh * GW + (2 * c + 2) * 32],
                                lhsT=q_sb[:, c, qcol:qcol + qm],
                                rhs=kz_sb[:, :, c,
                                          (g * 4 + j) * 32:(g * 4 + j) * 32 + 32],
                                tile_position=(0, 32 * j),
                                start=True, stop=False, skip_group_check=True,
                            )
                if ABLATE == "softmax":
                    abd_by_g[2 * gg] = abd_tiles[gg % 2]
                    abd_by_g[2 * gg + 1] = abd_tiles[gg % 2]
                    continue
                # ---- softmax along k, one op per step per 2 row groups.
                # t-pass: two 16-token fibers per row; the reduce is per
                # fiber block and the reciprocal is multiplied by a 0/1
                # fiber mask, which zeroes cross-fiber attention exactly
                # during the normalize (replaces the old additive -60
                # mask matmuls under the scores).
                a_sb = a_pool.tile([128, 2 * GW], BF16, tag="a")
                nc.scalar.activation(a_sb[:], sps[:],
                                     mybir.ActivationFunctionType.Exp)
                a4 = a_sb[:].rearrange("p (n f k) -> p n f k",
                                       n=2 * NH, f=nf)
                sums = sm_pool.tile([128, 2 * NH * nf], FP32, tag="sums")
                nc.vector.tensor_reduce(
                    sums[:], a4, axis=mybir.AxisListType.X,
                    op=mybir.AluOpType.add
                )
                recip = sm_pool.tile([128, 2 * NH * nf], FP32, tag="recip")
                nc.vector.reciprocal(recip[:], sums[:])
                if nf == 2:
                    recipm = sm_pool.tile([128, 2 * NH * nf], FP32,
                                          tag="recipm")
                    nc.vector.tensor_tensor(
                        recipm[:].rearrange("p (n f) -> p n f", n=2 * NH),
                        recip[:].rearrange("p (n f) -> p n f", n=2 * NH),
                        fm_sb[:].unsqueeze(1).broadcast_to((128, 2 * NH, nf)),
                        mybir.AluOpType.mult,
                    )
                    recip = recipm
                # normalize on GpSimd (measured ~4x faster than DVE for the
                # broadcast multiply), freeing VectorE for the transposes
                nc.gpsimd.tensor_tensor(
                    a4, a4,
                    recip[:].rearrange("p (n f) -> p n f", n=2 * NH)
                    .unsqueeze(3).broadcast_to((128, 2 * NH, nf, 32 // nf)),
                    mybir.AluOpType.mult,
                )
                # A -> A^T in place (DVE 32x32 block transpose, full width),
                # then per-row contiguous DVE copies into the block-diagonal
                # a_bd buffer: columns are row-major (j*512 + gh*256 +
                # head*32 + q), so each (j) slab is one (32, 512) copy.
                # Off-diagonal partitions stay zero from the one-time memset.
                at_sb = a_pool.tile([128, 2 * GW], BF16, tag="at")
                nc.vector.transpose(at_sb[:], a_sb[:])
                abd = abd_tiles[tc._abd_flip]
                tc._abd_flip ^= 1
                for j in range(4):
                    nc.vector.tensor_copy(
                        abd[32 * j:32 * (j + 1), 512 * j:512 * (j + 1)],
                        at_sb[32 * j:32 * (j + 1), :],
                    )
                abd_by_g[2 * gg] = abd
                abd_by_g[2 * gg + 1] = abd

        # ---- o^T = V^T A_bd, chunk-outer so only one o^T psum bank is live
        # at a time (the persistent abd buffers hold all 4 row groups);
        # evacuate each chunk to bf16 SBUF as soon as it completes
        ot_sb = ot_pool.tile([128, NCH, TILE], BF16)
        if ABLATE == "attn":
            for c in range(NCH):
                nc.gpsimd.tensor_copy(ot_sb[:, c, :], v_sb[:, c, :])
        else:
            for c in range(NCH):
                otp = ps_pool.tile([128, TILE], FP32, name="otp", tag="otp",
                                   bufs=2)
                for g in range(4):
                    gh = g % 2
                    abd4 = abd_by_g[g][:].rearrange("p (j x) -> p j x", j=4)
                    for p in range(2):
                        s0 = gh * GW + (2 * c + p) * 32
                        nc.tensor.matmul(
                            otp[64 * p:64 * (p + 1),
                                g * 4 * qm:(g + 1) * 4 * qm],
                            lhsT=v_sb[:, g,
                                      (2 * c + p) * 64:(2 * c + p + 1) * 64],
                            rhs=abd4[:, :, s0:s0 + qm],
                            tile_position=(0, 64 * p),
                        )
                if c % 2 == 0:
                    nc.scalar.copy(ot_sb[:, c, :], otp[:])
                else:
                    nc.vector.tensor_copy(ot_sb[:, c, :], otp[:])

        # ---- out-projection + emit / accumulate
        for mc in range(NCH):
            yps = ps_pool.tile([128, TILE], FP32, name="yps", tag="yps", bufs=2)
            for kc in range(NCH):
                nc.tensor.matmul(
                    yps[:],
                    lhsT=wo_sb[:, kc, 128 * mc:128 * (mc + 1)],
                    rhs=ot_sb[:, kc, :],
                    start=(kc == 0), stop=(kc == NCH - 1),
                )
            cs = slice(128 * mc, 128 * (mc + 1))
            if axis == "w":
                # first pass: write into the persistent SBUF accumulator,
                # folding the (summed) output bias in. w tile it covers
                # t=it, so this is the contiguous y_sb token range.
                nc.scalar.activation(
                    y_sb[:, mc, it * TILE:(it + 1) * TILE], yps[:],
                    mybir.ActivationFunctionType.Identity,
                    bias=bias_aps[mc],
                )
            elif axis == "t":
                # t tile it covers h-row `it`; psum tokens are (w 32, t 16)
                # t-fastest. Add the matching strided y_sb view (token order
                # (t, hl, w)) and emit contiguously in tile order.
                yv = (y_sb[:, mc, :].rearrange("p (t h w) -> p t h w",
                                               t=T, h=HL, w=W)[:, :, it, :]
                      .transpose([0, 2, 1]))                    # (128, w, t)
                ynew = y_pool.tile([128, W, T], BF16, tag="yt")
                yp3 = yps[:].rearrange("p (w t) -> p w t", w=W)
                nc.vector.tensor_tensor(
                    ynew[:], yv, yp3, mybir.AluOpType.add
                )
                nc.sync.dma_start(y_ap[cs, it * TILE:(it + 1) * TILE], ynew[:])
            else:
                # h-pass: independent token set; emit directly (no bias).
                y_sb2 = y_pool.tile([128, TILE], BF16, tag="yh")
                if mc % 2 == 0:
                    nc.scalar.copy(y_sb2[:], yps[:])
                else:
                    nc.vector.tensor_copy(y_sb2[:], yps[:])
                nc.sync.dma_start(y_ap[cs, it * TILE:(it + 1) * TILE], y_sb2[:])


def build_program():
    """Build + compile the SPMD bass program (same program on all 8 cores)."""
    nc = bacc.Bacc(
        "TRN2", target_bir_lowering=False, debug=False,
        enable_asserts=False, num_devices=N_CORES,
    )

    def din(name, shape, dt=BF16):
        return nc.dram_tensor(name, shape, dt, kind="ExternalInput").ap()

    x_w = din("x_w", (C, TOK_LOCAL))
    x_t = din("x_t", (C, TOK_LOCAL))
    x_h = din("x_h", (C, TOK_LOCAL))
    w_in = {}
    for ax in ("w", "t", "h"):
        for nm in ("wq", "wk", "wv", "wo"):
            w_in[f"{nm}_{ax}"] = din(f"{nm}_{ax}", (C, C))
    bias_in = din("bias", (C, 1), FP32)
    fm2_in = din("fm2", (128, 2), FP32)
    y_wt = nc.dram_tensor("y_wt", (C, TOK_LOCAL), BF16, kind="ExternalOutput").ap()
    y_h = nc.dram_tensor("y_h", (C, TOK_LOCAL), BF16, kind="ExternalOutput").ap()

    with tile.TileContext(nc) as tc:
        with contextlib.ExitStack() as ctx:
            xt_pool = ctx.enter_context(tc.tile_pool(name="xt", bufs=3))
            w_pool = ctx.enter_context(tc.tile_pool(name="wts", bufs=2))
            qk_pool = ctx.enter_context(tc.tile_pool(name="qk", bufs=2))
            v_pool = ctx.enter_context(tc.tile_pool(name="v", bufs=2))
            a_pool = ctx.enter_context(tc.tile_pool(name="a", bufs=3))
            sm_pool = ctx.enter_context(tc.tile_pool(name="sm", bufs=3))
            ot_pool = ctx.enter_context(tc.tile_pool(name="ot", bufs=2))
            y_pool = ctx.enter_context(tc.tile_pool(name="y", bufs=3))
            ps_pool = ctx.enter_context(tc.tile_pool(name="ps", bufs=2, space="PSUM"))
            sps_pool = ctx.enter_context(tc.tile_pool(name="sps", bufs=2, space="PSUM"))
            const_pool = ctx.enter_context(tc.tile_pool(name="const", bufs=1))

            # constants
            fm2_sb = const_pool.tile([128, 2], FP32)
            nc.sync.dma_start(fm2_sb[:], fm2_in[:])
            bias_sb = const_pool.tile([128, NCH], FP32)
            for mc in range(NCH):
                nc.sync.dma_start(
                    bias_sb[:, mc:mc + 1], bias_in[128 * mc:128 * (mc + 1), :]
                )
            bias_aps = [bias_sb[:, mc:mc + 1] for mc in range(NCH)]

            # persistent SBUF output accumulator for the w+t passes
            y_sb = const_pool.tile([128, NCH, TOK_LOCAL], BF16, name="y_sb")

            # persistent block-diagonal A^T buffers (double-buffered per
            # 2-rowgroup softmax), zeroed once
            abd_tiles = []
            for i in range(2):
                t = const_pool.tile([128, 4 * 512], BF16, name=f"abd{i}")
                nc.gpsimd.memset(t[:], 0.0)
                abd_tiles.append(t)
            tc._abd_flip = 0
            kz_tiles = []
            for i in range(2):
                t = const_pool.tile([128, 2, NCH, TILE], BF16, name=f"kz{i}")
                nc.gpsimd.memset(t[:], 0.0)
                kz_tiles.append(t)
            tc._kz_flip = 0

            pools = (xt_pool, qk_pool, v_pool, a_pool, sm_pool,
                     ot_pool, y_pool, ps_pool, sps_pool)

            for _rep in range(REPS):
              for ax, x_ap, y_ap in (("w", x_w, None), ("h", x_h, y_h),
                                     ("t", x_t, y_wt)):
                w_aps = []
                for nm in ("wq", "wk", "wv", "wo"):
                    wt = w_pool.tile([128, NCH, C], BF16, tag=nm, name=nm)
                    for kc in range(NCH):
                        nc.sync.dma_start(
                            wt[:, kc, :],
                            w_in[f"{nm}_{ax}"][128 * kc:128 * (kc + 1), :],
                        )
                    w_aps.append(wt)
                _build_pass(tc, pools, ax, x_ap, w_aps, y_ap, bias_aps, fm2_sb,
                            kz_tiles, abd_tiles, y_sb)

    nc.compile()
    return nc


_PROGRAM = None


def _get_program():
    global _PROGRAM
    if _PROGRAM is None:
        _PROGRAM = build_program()
    return _PROGRAM


def make_in_maps(inputs):
    """Host-side shard + layout prep: per-core input dicts."""
    x = np.asarray(inputs["x"], np.float32)          # (B, C, T, H, W)
    scale = 1.0 / np.sqrt(D)

    weights = {}
    for ax in ("w", "h", "t"):
        for nm in ("wq", "wk", "wv", "wo"):
            wm = np.asarray(inputs[f"{nm}_{ax}"], np.float32)
            if nm == "wq":
                wm = wm * scale
            # lhsT layout: (C_in, C_out) = W.T
            weights[f"{nm}_{ax}"] = np.ascontiguousarray(wm.T).astype(BF16_NP)
    bias = (np.asarray(inputs["bo_w"], np.float32)
            + np.asarray(inputs["bo_h"], np.float32)
            + np.asarray(inputs["bo_t"], np.float32)).reshape(C, 1)

    # t-pass fiber mask for the masked-reciprocal softmax: fm[p, f] = 1 iff
    # query partition p (= j*32 + q) belongs to 16-token fiber f of its row
    p = np.arange(128) % 32
    fm2 = np.stack([(p // 16) == f for f in range(2)], axis=1).astype(np.float32)

    in_maps = []
    for core in range(N_CORES):
        b, j = divmod(core, 2)
        xb = x[b]                                    # (C, T, H, W)
        xw = xb[:, :, 16 * j:16 * (j + 1), :]        # (C, T, HL, W) w-fastest
        xt = np.transpose(xw, (0, 2, 3, 1))          # (C, HL, W, T) t-fastest
        xh = np.transpose(xb[:, :, :, 16 * j:16 * (j + 1)],
                          (0, 1, 3, 2))              # (C, T, WL, H) h-fastest
        m = {
            "x_w": np.ascontiguousarray(xw).reshape(C, TOK_LOCAL).astype(BF16_NP),
            "x_t": np.ascontiguousarray(xt).reshape(C, TOK_LOCAL).astype(BF16_NP),
            "x_h": np.ascontiguousarray(xh).reshape(C, TOK_LOCAL).astype(BF16_NP),
            "bias": bias, "fm2": fm2,
        }
        m.update(weights)
        in_maps.append(m)
    return in_maps


def assemble_output(results):
    """Gather per-core y_wt/y_h partials into (B, C, T, H, W) fp32."""
    out = np.empty((B, C, T, H, W), np.float32)
    for core in range(N_CORES):
        b, j = divmod(core, 2)
        # y_wt tiles are h-rows: (C, HL, W, T) -> (C, T, HL, W)
        ywt = np.asarray(results[core]["y_wt"], np.float32).reshape(C, HL, W, T)
        out[b, :, :, 16 * j:16 * (j + 1), :] = np.transpose(ywt, (0, 3, 1, 2))
    for core in range(N_CORES):
        b, j = divmod(core, 2)
        # y_h tiles are t-slices: (C, T, WL, H) -> (C, T, H, WL)
        yh = np.asarray(results[core]["y_h"], np.float32).reshape(C, T, WL, H)
        out[b, :, :, :, 16 * j:16 * (j + 1)] += np.transpose(yh, (0, 1, 3, 2))
    return out


_RUNNER = None


def _get_runner():
    """Build the sharded PJRT callable once; reuse across kernel() calls."""
    global _RUNNER
    if _RUNNER is not None:
        return _RUNNER
    import jax
    from jax.sharding import Mesh, PartitionSpec
    from jax.experimental.shard_map import shard_map
    from concourse import bass2jax

    nc = _get_program()
    bass2jax.install_neuronx_cc_hook()
    partition_name = (nc.partition_id_tensor.name
                      if nc.partition_id_tensor else None)
    in_names, out_names, out_avals, zero_outs = [], [], [], []
    for alloc in nc.m.functions[0].allocations:
        if not isinstance(alloc, mybir.MemoryLocationSet):
            continue
        name = alloc.memorylocations[0].name
        if alloc.kind == "ExternalInput":
            if name != partition_name:
                in_names.append(name)
        elif alloc.kind == "ExternalOutput":
            out_names.append(name)
            shape = tuple(alloc.tensor_shape)
            dtype = mybir.dt.np(alloc.dtype)
            out_avals.append(jax.core.ShapedArray(shape, dtype))
            zero_outs.append(np.zeros((N_CORES * shape[0], *shape[1:]), dtype))
    n_params = len(in_names)
    all_in_names = list(in_names) + out_names
    if partition_name is not None:
        all_in_names.append(partition_name)

    def _body(*args):
        operands = list(args)
        if partition_name is not None:
            operands.append(bass2jax.partition_id_tensor())
        return tuple(bass2jax._bass_exec_p.bind(
            *operands,
            out_avals=tuple(out_avals),
            in_names=tuple(all_in_names),
            out_names=tuple(out_names),
            lowering_input_output_aliases=(),
            sim_require_finite=True,
            sim_require_nnan=True,
            nc=nc,
        ))

    devices = jax.devices()[:N_CORES]
    mesh = Mesh(np.asarray(devices), ("core",))
    in_specs = (PartitionSpec("core"),) * (n_params + len(out_names))
    out_specs = (PartitionSpec("core"),) * len(out_names)
    fn = jax.jit(shard_map(_body, mesh=mesh, in_specs=in_specs,
                           out_specs=out_specs, check_rep=False))

    def run(in_maps):
        concat_in = [
            np.concatenate([np.asarray(in_maps[c][nm]) for c in range(N_CORES)],
                           axis=0)
            for nm in in_names
        ]
        outs = fn(*concat_in, *zero_outs)
        return [
            {nm: np.asarray(outs[i]).reshape(N_CORES, *out_avals[i].shape)[c]
             for i, nm in enumerate(out_names)}
            for c in range(N_CORES)
        ]

    _RUNNER = run
    return run


def kernel(**inputs) -> np.ndarray:
    run = _get_runner()
    in_maps = make_in_maps(inputs)
    return assemble_output(run(in_maps))
